# revision 40
# baseline (speedup 1.0000x reference)
"""nn_GAT — 2-layer PyG-style GAT on 8 TRN2 NeuronCores (Bass/Tile).

Self-contained: kernel(**inputs) takes the FULL unsharded inputs
(as produced by setup_inputs) and returns the FULL [65536, 2] output.

Strategy (graph/data parallel, per sharding hint):
 - nodes sharded 8192/core; edges partitioned by dst core, grouped into
   128-dst-node groups; per 2-group batch the edge chunks are laid out
   [lo g0][lo g1][hi g0][hi g1] (lo/hi = permuted table row below/above
   32768, so int16 dma_gather indices suffice); all cores share one
   program (chunk counts maxed over cores).
 - dense phase: per-node 144B record [h fp8(128B) | a_s bf16(16B)] =
   x @ [W1 | W1@As] (attention folded in by linearity); a_d and x@Wp
   stay on-core in f32. The record table (256B-stride rows) is
   AllGathered in 4 row-slices fired as the GEMM produces the rows;
   tables are stored [slice][core][row] so each slice's collective
   output is one contiguous block (BIR requires contiguous outputs).
 - per-edge records are pulled by src via batched SWDGE dma_gather:
   4 calls per batch (half-blocks of the lo/hi runs) so all 4 queues
   carry equal descriptor counts — the queues' ~15-20ns/packet issue
   rate is this kernel's roofline. Segment softmax is shift-free (tiny
   logits); leaky-relu is computed as x + relu(-0.8x) with the relu on
   the Scalar engine (a DVE tensor_scalar measured ~4.2us/op on HW, and
   the HW Lrelu activation has a hardwired 0.01 slope). Messages are
   aggregated per dst group with host-built fp8 one-hot matmuls in PSUM.
 - layer 2: the per-edge a_d2 broadcast (one-hot-transpose matmuls) is
   computed during layer 1 while the transposed one-hot tiles are still
   resident (saves reloading 23MB of fp8 one-hot). The elu's "-1" is
   folded out: r2' = (relu(t)+exp(-relu(-t)))@M4 differs from the true
   record by colsum(M4); the a_s2/a_d2 shift folds into the lrelu/exp
   biases and the h2 shift into the host-side bias (attention weights
   sum to 1). The layer-2 record AllGather moves compact 16B rows in 4
   slices fired under layer 1's tail, each expanded on-core through
   SBUF into the 256B-stride gather table (gather elems read only the
   first 12B of each row, so the pad bytes are never touched).
"""
import numpy as np
import ml_dtypes

import concourse.bass as bass
import concourse.bacc as bacc
import concourse.mybir as mybir
import concourse.tile as tile
from concourse.masks import make_identity

F32 = mybir.dt.float32
BF16 = mybir.dt.bfloat16
F8 = mybir.dt.float8e4
U8 = mybir.dt.uint8
I16 = mybir.dt.int16
NEG_SLOPE = 0.2
BF = ml_dtypes.bfloat16
F8NP = ml_dtypes.float8_e4m3

N_NODES = 65536
IN_FEAT = 768
N_CORES = 8
HEADS = 8
C1 = 16
OUT_FEAT = 2
SUP = 512
HALF = 32768  # int16 split point for gather tables
NQ = 4        # SWDGE queues (Q7 core pairs) to spread gathers over
NCC = 8       # collective row-slices (overlap with producer phase; finer
              # slices shrink the serial last-slice tail before layer 2)
GB = 2        # dst groups per edge batch; chunks laid out [los..][his..]
              # per batch so each batch is two whole-block gather calls


class _Cfg:
    def __init__(self):
        self.N, self.IN, self.NC = N_NODES, IN_FEAT, N_CORES
        self.HEADS, self.C1, self.OUT, self.SUP = HEADS, C1, OUT_FEAT, SUP
        self.calo = []   # per-group lo-segment chunk counts
        self.cahi = []   # per-group hi-segment chunk counts

    @property
    def NL(self):
        return self.N // self.NC

    @property
    def G(self):
        return self.NL // 128

    @property
    def KT(self):
        return self.IN // 128

    @property
    def HID(self):
        return self.HEADS * self.C1

    @property
    def cg(self):
        return [a + b for a, b in zip(self.calo, self.cahi)]

    @property
    def Tlo(self):
        return sum(self.calo)

    @property
    def Thi(self):
        return sum(self.cahi)

    @property
    def T(self):
        return self.Tlo + self.Thi


def _wrap16(vals_by_slot, ncols):
    """vals_by_slot: int array indexed by slot j -> idx value.
    Returns [128, ncols] int16 with idx j at [j%16, j//16], replicated 8x."""
    a = np.zeros((16, ncols), np.int16)
    n = len(vals_by_slot)
    j = np.arange(n)
    a[j % 16, j // 16] = vals_by_slot
    return np.tile(a, (8, 1))


def _host_prep(cfg, x, edge_index, W1, att_src1, att_dst1, b1, Wp, bp,
               W2, att_src2, att_dst2, b2):
    N, NC, NL, G = cfg.N, cfg.NC, cfg.NL, cfg.G
    HID = cfg.HID
    x = np.asarray(x, np.float32)
    ei = np.asarray(edge_index)
    loops = np.arange(N, dtype=np.int64)
    src = np.concatenate([ei[0], loops]).astype(np.int64)
    dst = np.concatenate([ei[1], loops]).astype(np.int64)

    # the gathered tables are stored in [slice][core][row] order so each
    # AllGather row-slice lands as one contiguous output block (the BIR
    # verifier requires contiguous collective outputs); prow() maps a
    # global node id to its permuted table row, and the lo/hi int16 split
    # follows the PERMUTED row index (= local row < NL/2)
    QROWS = NL // NCC

    def prow(n):
        return (n % NL) // QROWS * (NC * QROWS) + (n // NL) * QROWS + n % QROWS

    hi_of = (src % NL) >= (NL // 2)
    # sort by (dst, hi) so each 128-dst group splits into lo/hi runs
    order = np.lexsort((hi_of, dst // 128))
    src, dst = src[order], dst[order]

    core_of = dst // NL
    core_starts = np.searchsorted(core_of, np.arange(NC + 1))

    # per-core per-group lo/hi counts
    nlo = np.zeros((NC, G), np.int64)
    nhi = np.zeros((NC, G), np.int64)
    for k in range(NC):
        s, e = core_starts[k], core_starts[k + 1]
        g = (dst[s:e] - k * NL) // 128
        hi = ((src[s:e] % NL) >= (NL // 2)).astype(np.int64)
        nlo[k] = np.bincount(g, weights=1 - hi, minlength=G)
        nhi[k] = np.bincount(g, weights=hi, minlength=G)
    calo = np.maximum((-(-nlo.astype(np.int64) // 128)).max(axis=0), 1)
    cahi = np.maximum((-(-nhi.astype(np.int64) // 128)).max(axis=0), 1)
    cfg.calo = [int(c) for c in calo]
    cfg.cahi = [int(c) for c in cahi]
    cg = calo + cahi
    offlo = np.concatenate([[0], np.cumsum(calo)])
    offhi = np.concatenate([[0], np.cumsum(cahi)])
    off = np.concatenate([[0], np.cumsum(cg)])
    Tlo, Thi, T = int(calo.sum()), int(cahi.sum()), int(cg.sum())

    W1 = np.asarray(W1, np.float32)
    Wp = np.asarray(Wp, np.float32)
    As = np.zeros((HID, cfg.HEADS), np.float32)
    Ad = np.zeros((HID, cfg.HEADS), np.float32)
    hh = np.repeat(np.arange(cfg.HEADS), cfg.C1)
    As[np.arange(HID), hh] = np.asarray(att_src1, np.float32).ravel()
    Ad[np.arange(HID), hh] = np.asarray(att_dst1, np.float32).ravel()
    WBIG = np.concatenate([W1, W1 @ As, W1 @ Ad, Wp], axis=1)  # [IN, 272]

    W2 = np.asarray(W2, np.float32)
    M4 = np.concatenate(
        [W2, W2 @ np.asarray(att_src2, np.float32).T,
         W2 @ np.asarray(att_dst2, np.float32).T], axis=1)     # [HID, 4]
    BC1 = (np.asarray(b1, np.float32) + np.asarray(bp, np.float32))[None, :]
    B2 = np.asarray(b2, np.float32)
    # elu(-1) fold: r2' = h2sum@M4 = r2_true + colsum(M4). as2/ad2 shift is
    # folded into the layer-2 Lrelu bias; h2 shift into the host bias.
    csum = M4.sum(axis=0)                                      # [4]
    d = -(csum[2] + csum[3])          # shift to apply before layer-2 lrelu
    # lrelu(x+d) computed as (x+d) + relu(-0.8(x+d)); col0 biases the Exp,
    # col1 biases the Relu (HW Lrelu has a hardwired 0.01 slope, unusable)
    DLT = np.array([[d, -0.8 * d]], np.float32)
    B2eff = B2 - csum[0:2]

    in_maps = []
    for k in range(NC):
        s, e = core_starts[k], core_starts[k + 1]
        sk, dk = src[s:e], dst[s:e]
        gk = (dk - k * NL) // 128
        hik = (sk % NL) >= (NL // 2)
        # rank within (group, seg): edges are sorted by (group, hi) so
        # positions within each (g, seg) run are consecutive
        pos = np.arange(e - s)
        seg_key = gk * 2 + hik
        seg_start = np.concatenate([[0], np.cumsum(np.bincount(
            seg_key, minlength=2 * G))])
        r_in_seg = pos - seg_start[seg_key]
        # chunk position within the batch: [lo g0][lo g1][hi g0][hi g1]
        g0b = (gk // GB) * GB               # first group of the edge's batch
        lo_base = np.where(gk == g0b, 0, calo[g0b])
        lsum = calo[g0b] + calo[g0b + 1]
        hi_base = lsum + np.where(gk == g0b, 0, cahi[g0b])
        cpos = np.where(hik, hi_base, lo_base) + r_in_seg // 128
        chunk = off[g0b] + cpos             # global chunk index
        lane = r_in_seg % 128

        # lo/hi gather slot (position within that segment's idx stream)
        slot_lo = (offlo[gk] * 128 + r_in_seg)[~hik]
        slot_hi = (offhi[gk] * 128 + r_in_seg)[hik]
        vals = np.zeros(Tlo * 128, np.int64)
        vals[slot_lo] = prow(sk[~hik])
        SRCLO = _wrap16(vals, 8 * Tlo)
        vals = np.zeros(Thi * 128, np.int64)
        vals[slot_hi] = prow(sk[hik]) - HALF
        SRCHI = _wrap16(vals, 8 * Thi)
        # one-hot tiles (static graph): chunk c col-block holds
        # oh[lane, dstpos] = 1 for each real edge; padding rows all-zero,
        # plus the transposed tiles for dst->edge broadcasts (a_d terms)
        dpos = (dk - k * NL) % 128
        OH = np.zeros((128, T * 128), F8NP)
        OH[lane, chunk * 128 + dpos] = 1
        OHT = np.zeros((128, T * 128), F8NP)
        OHT[dpos, chunk * 128 + lane] = 1

        in_maps.append({
            "XT": np.ascontiguousarray(
                x[k * NL:(k + 1) * NL].T).astype(BF),
            "WBIG": WBIG.astype(BF), "M4": M4, "BC1": BC1, "DLT": DLT,
            "SRCLO": SRCLO, "SRCHI": SRCHI, "OH": OH, "OHT": OHT,
        })
    return cfg, in_maps, B2eff


def _unshard(cfg, outs, B2eff):
    parts = []
    for k in range(cfg.NC):
        o = outs[k]["OUT"]                       # [4, G*128]
        num = o[0:2].reshape(2, cfg.G, 128)
        den = o[2].reshape(cfg.G, 128)
        r = (num / den[None]).transpose(1, 2, 0).reshape(cfg.NL, 2)
        parts.append(r + B2eff[None, :])
    return np.concatenate(parts, axis=0).astype(np.float32)


MAX_GCH = 32  # chunks (x128 idxs) per dma_gather call


def _gather(nc, out_tile, out_col0, in_ap, idx_tile, idx_col0, nchunks, elem,
            stride_bytes, queue_num):
    """Batched dma_gather, split into <=MAX_GCH-chunk calls on queue_num.
    out rows: [128, nchunks, elem] at out_tile cols out_col0*elem;
    idx cols: idx_tile[:, 8*idx_col0 : 8*(idx_col0+nchunks)]."""
    eng = nc.gpsimd
    insts = []
    for c0 in range(0, nchunks, MAX_GCH):
        nch = min(MAX_GCH, nchunks - c0)
        o = out_tile[:, (out_col0 + c0) * elem:(out_col0 + c0 + nch) * elem]
        o = o.rearrange("p (n e) -> p n e", e=elem)
        ix = idx_tile[:, 8 * (idx_col0 + c0):8 * (idx_col0 + c0 + nch)]
        insts.append(eng.add_instruction(
            mybir.InstDMAGatherAnt(
                name=nc.get_next_instruction_name(),
                ins=[*eng.lower_ap_dma(in_ap, for_custom_bir_dma=True),
                     eng.lower_ap(ix),
                     eng.lower_val_access(eng.to_reg(nch * 128))],
                outs=[eng.lower_ap(o)],
                transpose=False,
                num_idxs=nch * 128,
                elem_size=elem,
                stride_bytes_256=stride_bytes // 256,
                gen_mode=0,
                single_packet=False,
                queue_num=queue_num,
            )))
    return insts


def _build(cfg):
    NC = cfg.NC
    NL, G, KT = cfg.NL, cfg.G, cfg.KT
    H, C1_, HID, OUT = cfg.HEADS, cfg.C1, cfg.HID, cfg.OUT
    REC = HID + 2 * H        # 144 (gemm out: h|a_s|a_d)
    GREC = HID + H           # 136 (gathered: h|a_s)
    RBYTES = HID + 2 * H     # 144B gathered record: h fp8 | a_s bf16
    TBYTES = 256             # T1main row stride in bytes
    R2W = 64                 # f32 per R2main row (256B)
    SUP_ = cfg.SUP
    calo, cahi, cg = cfg.calo, cfg.cahi, cfg.cg
    Tlo, Thi, T = cfg.Tlo, cfg.Thi, cfg.T
    offlo = [0]
    for c in calo:
        offlo.append(offlo[-1] + c)
    offhi = [0]
    for c in cahi:
        offhi.append(offhi[-1] + c)
    off = [0]
    for c in cg:
        off.append(off[-1] + c)

    QROWS = NL // NCC        # rows per collective slice (2048)
    QG = G // NCC            # groups per collective slice (16)

    qrot = [0]

    def nextq():
        q = qrot[0]
        qrot[0] = (q + 1) % NQ
        return q

    nc = bacc.Bacc("TRN2", target_bir_lowering=False, debug=False,
                   num_devices=NC, num_swdge_queues=NQ)
    XT = nc.dram_tensor("XT", [cfg.IN, NL], BF16, kind="ExternalInput")
    WBIGd = nc.dram_tensor("WBIG", [cfg.IN, REC + HID], BF16, kind="ExternalInput")
    M4d = nc.dram_tensor("M4", [HID, 4], F32, kind="ExternalInput")
    BC1d = nc.dram_tensor("BC1", [1, HID], F32, kind="ExternalInput")
    DLTd = nc.dram_tensor("DLT", [1, 2], F32, kind="ExternalInput")
    SRCLOd = nc.dram_tensor("SRCLO", [128, 8 * Tlo], I16, kind="ExternalInput")
    SRCHId = nc.dram_tensor("SRCHI", [128, 8 * Thi], I16, kind="ExternalInput")
    OHd = nc.dram_tensor("OH", [128, T * 128], F8, kind="ExternalInput")
    OHTd = nc.dram_tensor("OHT", [128, T * 128], F8, kind="ExternalInput")
    OUTd = nc.dram_tensor("OUT", [4, G * 128], F32, kind="ExternalOutput")

    with tile.TileContext(nc) as tc:
        with (
            tc.tile_pool(name="dram", bufs=1, space="DRAM") as dram,
            tc.tile_pool(name="const", bufs=1) as cb,
            tc.tile_pool(name="persist", bufs=1) as pp,
        ):
            T1locq = [dram.tile([QROWS, TBYTES], U8, name=f"T1locq{q}")
                      for q in range(NCC)]
            T1main = dram.tile([cfg.N, TBYTES], U8)
            R2locq = [dram.tile([QROWS, 4], F32, name=f"R2locq{q}")
                      for q in range(NCC)]
            R2comp = dram.tile([cfg.N, 4], F32)
            R2allcP = dram.tile([cfg.N, R2W], F32)

            ident = cb.tile([128, 128], F32)
            make_identity(nc, ident[:])
            wb_sb = []
            for kk in range(KT):
                t = cb.tile([128, REC + HID], BF16, tag=f"wb{kk}", name=f"wb{kk}")
                nc.sync.dma_start(t[:], WBIGd[kk * 128:(kk + 1) * 128, :])
                wb_sb.append(t)
            m4_sb = cb.tile([HID, 4], F32)
            nc.sync.dma_start(m4_sb[:], M4d[:])
            ones1 = cb.tile([1, 128], F32)
            nc.vector.memset(ones1[:], 1.0)
            bc1row = cb.tile([1, HID], F32)
            nc.sync.dma_start(bc1row[:], BC1d[:])
            dltrow = cb.tile([1, 2], F32)
            nc.sync.dma_start(dltrow[:], DLTd[:])
            with tc.tile_pool(name="bpsum", bufs=1, space="PSUM") as bps:
                bp1 = bps.tile([128, HID], F32)
                nc.tensor.matmul(bp1[:], lhsT=ones1[:], rhs=bc1row[:], start=True, stop=True)
                BC1T = cb.tile([128, HID], F32)
                nc.vector.tensor_copy(BC1T[:], bp1[:])
                bpd = bps.tile([128, 2], F32, tag="bpd", name="bpd")
                nc.tensor.matmul(bpd[:], lhsT=ones1[:], rhs=dltrow[:], start=True, stop=True)
                DLTT = cb.tile([128, 2], F32)
                nc.vector.tensor_copy(DLTT[:], bpd[:])

            p_sb = pp.tile([128, G * HID], F32)
            ad_sb = pp.tile([128, G * H], BF16)
            r2stage = pp.tile([128, G * 4], F32)
            adp2all = pp.tile([128, T], F32)
            ad2bf = pp.tile([128, G], BF16)
            outstage = pp.tile([4, G * 128], F32)
            nc.vector.memset(outstage[:], 0.0)
            # gather index tables stay resident (shared by both layers)
            silo_all = pp.tile([128, 8 * Tlo], I16)
            nc.sync.dma_start(silo_all[:], SRCLOd[:])
            sihi_all = pp.tile([128, 8 * Thi], I16)
            nc.sync.dma_start(sihi_all[:], SRCHId[:])

            # ---------------- phase A: GEMM (+ T1 AllGather slices)
            n_sup = NL // SUP_
            m_per = SUP_ // 128
            sup_per_q = n_sup // NCC
            ccs1 = []
            with (
                tc.tile_pool(name="xts", bufs=2 * KT) as xp,
                tc.tile_pool(name="gpsum", bufs=3, space="PSUM") as gps,
                tc.tile_pool(name="grec", bufs=3) as grp,
            ):
                for s in range(n_sup):
                    xts = []
                    for kk in range(KT):
                        t = xp.tile([128, SUP_], BF16, tag="xts", name="xts")
                        nc.sync.dma_start(
                            t[:], XT[kk * 128:(kk + 1) * 128, s * SUP_:(s + 1) * SUP_])
                        xts.append(t)
                    for m in range(m_per):
                        gm = s * m_per + m
                        q = gm // QG
                        r0 = (gm % QG) * 128
                        ps = gps.tile([128, REC + HID], F32, tag="gp", name="gp")
                        for kk in range(KT):
                            nc.tensor.matmul(
                                ps[:], lhsT=xts[kk][:, m * 128:(m + 1) * 128],
                                rhs=wb_sb[kk][:], start=(kk == 0), stop=(kk == KT - 1))
                        rec = grp.tile([128, TBYTES], U8, tag="rec", name="rec")
                        nc.vector.tensor_copy(
                            rec[:, 0:HID].bitcast(F8), ps[:, 0:HID])
                        nc.vector.tensor_copy(
                            rec[:, HID:RBYTES].bitcast(BF16), ps[:, HID:GREC])
                        nc.vector.tensor_copy(
                            ad_sb[:, gm * H:(gm + 1) * H], ps[:, GREC:REC])
                        nc.vector.tensor_copy(
                            p_sb[:, gm * HID:(gm + 1) * HID], ps[:, REC:REC + HID])
                        nc.sync.dma_start(T1locq[q][r0:r0 + 128, :], rec[:])
                    # fire the AllGather slice as soon as its rows are done
                    # (T1main is in [slice][core][row] order so each slice's
                    # output is one contiguous block)
                    if (s + 1) % sup_per_q == 0:
                        q = (s + 1) // sup_per_q - 1
                        out_ap = bass.AP(
                            T1main.tensor, q * NC * QROWS * TBYTES,
                            [[TBYTES, NC * QROWS], [1, TBYTES]])
                        ccs1.append(nc.gpsimd.collective_compute(
                            "AllGather", mybir.AluOpType.bypass,
                            replica_groups=[list(range(NC))],
                            ins=[T1locq[q].opt()], outs=[out_ap.opt()]))

            T1lo_h = T1main[:][0:HALF, 0:RBYTES]
            T1hi_h = T1main[:][HALF:cfg.N, 0:RBYTES]

            # ---------------- phase C: layer-1 edge pass + layer-2 prep
            assert G % GB == 0
            ccs2 = []

            def _cpos(gs):
                """Batch-relative chunk positions per group for the
                [lo g0][lo g1][hi g0][hi g1] batch layout."""
                c0, h0 = calo[gs[0]], cahi[gs[0]]
                c1, h1 = calo[gs[1]], cahi[gs[1]]
                L = c0 + c1
                return {
                    gs[0]: list(range(0, c0)) + list(range(L, L + h0)),
                    gs[1]: list(range(c0, L)) + list(range(L + h0, L + h0 + h1)),
                }

            SUBR = 2048  # rows per expansion sub-chunk (16 rows/partition)
            rp_insts = []

            def _fire_r2(q):
                r2v = r2stage[:, q * QG * 4:(q + 1) * QG * 4].rearrange(
                    "p (g r) -> p g r", r=4)
                nc.sync.dma_start(
                    R2locq[q][:].rearrange("(g p) r -> p g r", p=128), r2v)
                out_ap = bass.AP(
                    R2comp.tensor, q * NC * QROWS * 4,
                    [[4, NC * QROWS], [1, 4]])
                ccs2.append(nc.gpsimd.collective_compute(
                    "AllGather", mybir.AluOpType.bypass,
                    replica_groups=[list(range(NC))],
                    ins=[R2locq[q].opt()], outs=[out_ap.opt()]))
                # expand the compact slice into 256B-stride gather rows via
                # SBUF (the 12B gather elems never read the pad bytes, so no
                # zero-fill needed); contiguous DMAs = 128 descriptors each
                base = q * NC * QROWS
                for sb in range(NC * QROWS // SUBR):
                    r0 = base + sb * SUBR
                    cst = xpn.tile([128, SUBR // 128 * 4], F32,
                                   tag="cst", name="cst")
                    ld = nc.sync.dma_start(
                        cst[:].rearrange("p (g r) -> p g r", r=4),
                        R2comp[:][r0:r0 + SUBR, :].rearrange(
                            "(p g) r -> p g r", p=128))
                    tile.add_dep_helper(ld.ins, ccs2[-1].ins, sync=True,
                                        reason="xp")
                    est = xpn.tile([128, SUBR // 128 * R2W], F32,
                                   tag="est", name="est")
                    nc.vector.tensor_copy(
                        bass.AP(est.tensor, est[:].offset,
                                [est[:].ap[0], [R2W, SUBR // 128], [1, 4]]),
                        cst[:].rearrange("p (g r) -> p g r", r=4))
                    rp_insts.append(nc.sync.dma_start(
                        R2allcP[:][r0:r0 + SUBR, :].rearrange(
                            "(p g) r -> p g r", p=128), est[:].rearrange(
                                "p (g r) -> p g r", r=R2W)))

            with (
                tc.tile_pool(name="xpnd", bufs=2) as xpn,
                tc.tile_pool(name="erec", bufs=3) as ep,
                tc.tile_pool(name="ework", bufs=2) as ew,
                tc.tile_pool(name="escall", bufs=3) as esc,
                tc.tile_pool(name="eoh", bufs=2) as eoh,
                tc.tile_pool(name="epsum", bufs=2, space="PSUM") as eps,
                tc.tile_pool(name="apsum", bufs=2, space="PSUM") as aps,
                tc.tile_pool(name="tpsum", bufs=2, space="PSUM") as tps,
            ):
                for g0 in range(0, G, GB):
                    gs = list(range(g0, min(g0 + GB, G)))
                    nb = off[gs[-1] + 1] - off[g0]
                    base0 = off[g0]
                    oht = eoh.tile([128, nb * 128], F8, tag="oht", name="oht")
                    nc.sync.dma_start(
                        oht[:], OHd[:, base0 * 128:(base0 + nb) * 128])
                    ohtT = eoh.tile([128, nb * 128], F8, tag="ohtT", name="ohtT")
                    nc.sync.dma_start(
                        ohtT[:], OHTd[:, base0 * 128:(base0 + nb) * 128])
                    # one whole-block gather call per (batch, lo/hi table)
                    cpos = _cpos(gs)
                    nlo_b = calo[gs[0]] + calo[gs[1]]
                    nhi_b = nb - nlo_b
                    hall = ep.tile([128, nb * RBYTES], U8, tag="hall", name="hall")
                    # two half-block calls per table so each batch spreads
                    # its descriptors across all 4 SWDGE queues
                    hlo = (nlo_b + 1) // 2
                    hhi = (nhi_b + 1) // 2
                    for c0, nch, tbl, idxt, ic0 in (
                            (0, hlo, T1lo_h, silo_all, offlo[gs[0]]),
                            (hlo, nlo_b - hlo, T1lo_h, silo_all,
                             offlo[gs[0]] + hlo),
                            (nlo_b, hhi, T1hi_h, sihi_all, offhi[gs[0]]),
                            (nlo_b + hhi, nhi_b - hhi, T1hi_h, sihi_all,
                             offhi[gs[0]] + hhi)):
                        for gi in _gather(nc, hall[:], c0, tbl, idxt[:], ic0,
                                          nch, RBYTES, TBYTES, nextq()):
                            for cc in ccs1:
                                tile.add_dep_helper(gi.ins, cc.ins, sync=True,
                                                    reason="ag1")
                    # batch-wide per-edge a_d via one-hot-transpose matmuls;
                    # tail region [nb*H:nb*H+nb] is layer-2's a_d2 (filled
                    # after ps4 below, same PSUM bank)
                    adp = aps.tile([128, nb * H + nb], F32, tag="adp", name="adp")
                    for g in gs:
                        for c in cpos[g]:
                            nc.tensor.matmul(
                                adp[:, c * H:(c + 1) * H],
                                lhsT=ohtT[:, c * 128:(c + 1) * 128],
                                rhs=ad_sb[:, g * H:(g + 1) * H],
                                start=True, stop=True)
                    # batch-wide softmax numerators: e = lrelu(a_s + a_d);
                    # exp on Scalar engine straight into the scall tile
                    scall = esc.tile([128, nb * GREC], BF16, tag="scall", name="scall")
                    as_ap = bass.AP(
                        hall.tensor, hall[:].offset + HID,
                        [hall[:].ap[0], [RBYTES, nb], [1, 2 * H]]).bitcast(BF16)
                    ad_ap = bass.AP(
                        adp.tensor, adp[:].offset,
                        [adp[:].ap[0], [H, nb], [1, H]])
                    epre = ew.tile([128, nb * H], F32, tag="epre", name="epre")
                    nc.vector.tensor_tensor(
                        out=epre[:].rearrange("p (n h) -> p n h", h=H),
                        in0=as_ap, in1=ad_ap, op=mybir.AluOpType.add)
                    # lrelu(x) = x + relu(-0.8x); HW Lrelu slope is fixed 0.01
                    rneg = ew.tile([128, nb * H], F32, tag="rneg", name="rneg")
                    nc.scalar.activation(rneg[:], epre[:],
                                         mybir.ActivationFunctionType.Relu,
                                         scale=-(1.0 - NEG_SLOPE))
                    lrm = ew.tile([128, nb * H], F32, tag="lrm", name="lrm")
                    nc.vector.tensor_add(lrm[:], epre[:], rneg[:])
                    ex_ap = bass.AP(
                        scall.tensor, scall[:].offset + HID,
                        [scall[:].ap[0], [GREC, nb], [1, H]])
                    nc.scalar.activation(
                        ex_ap, lrm[:].rearrange("p (n h) -> p n h", h=H),
                        mybir.ActivationFunctionType.Exp)
                    # scaled messages (bf16 h x bf16 ex -> bf16)
                    out4 = bass.AP(
                        scall.tensor, scall[:].offset,
                        [scall[:].ap[0], [GREC, nb], [C1_, H], [1, C1_]])
                    in04 = bass.AP(
                        hall.tensor, hall[:].offset,
                        [hall[:].ap[0], [RBYTES, nb], [C1_, H], [1, C1_]]).bitcast(F8)
                    in14 = bass.AP(
                        scall.tensor, scall[:].offset + HID,
                        [scall[:].ap[0], [GREC, nb], [1, H], [0, C1_]])
                    nc.vector.tensor_tensor(
                        out=out4, in0=in04, in1=in14, op=mybir.AluOpType.mult)

                    for g in gs:
                        psg = eps.tile([128, GREC], F32, tag="psg", name="psg")
                        for j, c in enumerate(cpos[g]):
                            nc.tensor.matmul(
                                psg[:], lhsT=oht[:, c * 128:(c + 1) * 128],
                                rhs=scall[:, c * GREC:(c + 1) * GREC],
                                start=(j == 0), stop=(j == len(cpos[g]) - 1))
                        # normalize + residual + elu(+1) -> h2sum -> r2 records
                        rec8 = ew.tile([128, H], F32, tag="rec8", name="rec8")
                        nc.vector.reciprocal(rec8[:], psg[:, HID:GREC])
                        t1 = ew.tile([128, HID], F32, tag="t1", name="t1")
                        nc.vector.tensor_tensor(
                            out=t1[:].rearrange("p (h c) -> p h c", h=H),
                            in0=psg[:, 0:HID].rearrange("p (h c) -> p h c", h=H),
                            in1=rec8[:].to_broadcast([128, H, C1_]),
                            op=mybir.AluOpType.mult)
                        nc.vector.tensor_add(t1[:], t1[:], p_sb[:, g * HID:(g + 1) * HID])
                        nc.vector.tensor_add(t1[:], t1[:], BC1T[:])
                        tmin = ew.tile([128, HID], F32, tag="tmin", name="tmin")
                        nc.scalar.activation(tmin[:], t1[:],
                                             mybir.ActivationFunctionType.Relu,
                                             scale=-1.0)
                        texp = ew.tile([128, HID], F32, tag="texp", name="texp")
                        nc.scalar.activation(texp[:], tmin[:],
                                             mybir.ActivationFunctionType.Exp,
                                             scale=-1.0)
                        tmax = ew.tile([128, HID], F32, tag="tmax", name="tmax")
                        nc.scalar.activation(tmax[:], t1[:],
                                             mybir.ActivationFunctionType.Relu)
                        h2sum = ew.tile([128, HID], F32, tag="h2sum", name="h2sum")
                        nc.vector.tensor_add(h2sum[:], texp[:], tmax[:])
                        pst = tps.tile([128, HID], F32, tag="pst", name="pst")
                        nc.tensor.transpose(pst[:], h2sum[:], ident[:])
                        tT = ew.tile([128, HID], F32, tag="tT", name="tT")
                        nc.vector.tensor_copy(tT[:], pst[:])
                        ps4 = tps.tile([128, 4], F32, tag="ps4", name="ps4")
                        nc.tensor.matmul(ps4[:], lhsT=tT[:], rhs=m4_sb[:],
                                         start=True, stop=True)
                        nc.vector.tensor_copy(r2stage[:, g * 4:(g + 1) * 4], ps4[:])
                        # layer-2 prep while ohtT is resident: a_d2 broadcast
                        nc.vector.tensor_copy(ad2bf[:, g:g + 1], ps4[:, 3:4])
                        for c in cpos[g]:
                            nc.tensor.matmul(
                                adp[:, nb * H + c:nb * H + c + 1],
                                lhsT=ohtT[:, c * 128:(c + 1) * 128],
                                rhs=ad2bf[:, g:g + 1],
                                start=True, stop=True)
                    nc.vector.tensor_copy(
                        adp2all[:, base0:base0 + nb], adp[:, nb * H:nb * H + nb])

                    # fire R2 AllGather slices 2 batches after their groups
                    # retire so the queue-head waits are already satisfied
                    # (an early sem-wait would stall the issuing queues)
                    gend = gs[-1] + 1
                    while (len(ccs2) < NCC
                           and gend >= QG * (len(ccs2) + 1) + 2 * GB):
                        _fire_r2(len(ccs2))
                while len(ccs2) < NCC:
                    _fire_r2(len(ccs2))

            R2lo_h = R2allcP[:][0:HALF, 0:3]
            R2hi_h = R2allcP[:][HALF:cfg.N, 0:3]

            # ---------------- phase E: layer-2 edge pass
            with (
                tc.tile_pool(name="e2rec", bufs=3) as ep2,
                tc.tile_pool(name="e2work", bufs=3) as ew2,
                tc.tile_pool(name="e2sc", bufs=3) as esc2,
                tc.tile_pool(name="e2oh", bufs=3) as eoh2,
                tc.tile_pool(name="e2psum", bufs=2, space="PSUM") as eps2,
            ):
                for g0 in range(0, G, GB):
                    gs = list(range(g0, min(g0 + GB, G)))
                    nb = off[gs[-1] + 1] - off[g0]
                    base0 = off[g0]
                    oht2 = eoh2.tile([128, nb * 128], F8, tag="oht2", name="oht2")
                    nc.sync.dma_start(
                        oht2[:], OHd[:, base0 * 128:(base0 + nb) * 128])
                    cpos = _cpos(gs)
                    nlo_b = calo[gs[0]] + calo[gs[1]]
                    nhi_b = nb - nlo_b
                    rall = ep2.tile([128, nb * 3], F32, tag="rall", name="rall")
                    hlo = (nlo_b + 1) // 2
                    hhi = (nhi_b + 1) // 2
                    for c0, nch, tbl, idxt, ic0 in (
                            (0, hlo, R2lo_h, silo_all, offlo[gs[0]]),
                            (hlo, nlo_b - hlo, R2lo_h, silo_all,
                             offlo[gs[0]] + hlo),
                            (nlo_b, hhi, R2hi_h, sihi_all, offhi[gs[0]]),
                            (nlo_b + hhi, nhi_b - hhi, R2hi_h, sihi_all,
                             offhi[gs[0]] + hhi)):
                        for gi in _gather(nc, rall[:], c0, tbl, idxt[:], ic0,
                                          nch, 3, R2W * 4, nextq()):
                            for st in rp_insts:
                                tile.add_dep_helper(gi.ins, st.ins, sync=True,
                                                    reason="rp2")
                    sc2 = esc2.tile([128, nb * 3], BF16, tag="sc2", name="sc2")
                    as2_ap = bass.AP(
                        rall.tensor, rall[:].offset + 2,
                        [rall[:].ap[0], [3, nb], [1, 1]])
                    ad2_ap = bass.AP(
                        adp2all.tensor, adp2all[:].offset + base0,
                        [adp2all[:].ap[0], [1, nb], [1, 1]])
                    epre = ew2.tile([128, nb], F32, tag="ep2", name="ep2")
                    nc.vector.tensor_tensor(
                        out=epre[:].rearrange("p (n h) -> p n h", h=1),
                        in0=as2_ap, in1=ad2_ap, op=mybir.AluOpType.add)
                    # lrelu(x+d) = (x+d) + relu(-0.8(x+d)); the relu bias is
                    # -0.8d (DLTT col1), the +d rides on the Exp bias (col0)
                    rneg = ew2.tile([128, nb], F32, tag="rn2", name="rn2")
                    nc.scalar.activation(rneg[:], epre[:],
                                         mybir.ActivationFunctionType.Relu,
                                         scale=-(1.0 - NEG_SLOPE),
                                         bias=DLTT[:, 1:2])
                    lrm = ew2.tile([128, nb], F32, tag="lrm2", name="lrm2")
                    nc.vector.tensor_add(lrm[:], epre[:], rneg[:])
                    ex_ap = bass.AP(
                        sc2.tensor, sc2[:].offset + 2,
                        [sc2[:].ap[0], [3, nb], [1, 1]])
                    nc.scalar.activation(
                        ex_ap, lrm[:].rearrange("p (n h) -> p n h", h=1),
                        mybir.ActivationFunctionType.Exp, bias=DLTT[:, 0:1])
                    out4 = bass.AP(
                        sc2.tensor, sc2[:].offset,
                        [sc2[:].ap[0], [3, nb], [1, 1], [1, OUT]])
                    in04 = bass.AP(
                        rall.tensor, rall[:].offset,
                        [rall[:].ap[0], [3, nb], [1, 1], [1, OUT]])
                    in14 = bass.AP(
                        sc2.tensor, sc2[:].offset + 2,
                        [sc2[:].ap[0], [3, nb], [1, 1], [0, OUT]])
                    nc.vector.tensor_tensor(
                        out=out4, in0=in04, in1=in14, op=mybir.AluOpType.mult)

                    for g in gs:
                        ps2 = eps2.tile([3, 128], F32, tag="ps2", name="ps2")
                        for j, c in enumerate(cpos[g]):
                            nc.tensor.matmul(
                                ps2[:], lhsT=sc2[:, c * 3:(c + 1) * 3],
                                rhs=oht2[:, c * 128:(c + 1) * 128],
                                start=(j == 0), stop=(j == len(cpos[g]) - 1))
                        nc.vector.tensor_copy(
                            outstage[0:3, g * 128:(g + 1) * 128], ps2[:])
                nc.sync.dma_start(OUTd[:], outstage[:])

    nc.compile()
    return nc


_CACHE = {}


def kernel(x, edge_index, W1, att_src1, att_dst1, b1, Wp, bp,
           W2, att_src2, att_dst2, b2, _trace=False):
    from concourse.bass_utils import run_bass_kernel_spmd
    cfg = _Cfg()
    cfg, in_maps, B2eff = _host_prep(
        cfg, x, edge_index, W1, att_src1, att_dst1, b1, Wp, bp,
        W2, att_src2, att_dst2, b2)
    key = (tuple(cfg.calo), tuple(cfg.cahi))
    if key not in _CACHE:
        _CACHE[key] = _build(cfg)
    nc = _CACHE[key]
    res = run_bass_kernel_spmd(
        nc, in_maps, core_ids=list(range(cfg.NC)), trace=_trace)
    out = _unshard(cfg, [res.results[k] for k in range(cfg.NC)], B2eff)
    kernel.last_exec_time_ns = res.exec_time_ns
    return out


# revision 42
# speedup vs baseline: 1.0635x; 1.0635x over previous
"""nn_GAT — 2-layer PyG-style GAT on 8 TRN2 NeuronCores (Bass/Tile).

Self-contained: kernel(**inputs) takes the FULL unsharded inputs
(as produced by setup_inputs) and returns the FULL [65536, 2] output.

Strategy (graph/data parallel, per sharding hint):
 - nodes sharded 8192/core; edges partitioned by dst core, grouped into
   128-dst-node groups; per 2-group batch the edge chunks are laid out
   [lo g0][lo g1][hi g0][hi g1] (lo/hi = permuted table row below/above
   32768, so int16 dma_gather indices suffice); all cores share one
   program (chunk counts maxed over cores).
 - dense phase: per-node 144B record [h fp8(128B) | a_s bf16(16B)] =
   x @ [W1 | W1@As] (attention folded in by linearity); a_d and x@Wp
   stay on-core in f32. The record table (256B-stride rows) is
   AllGathered in 4 row-slices fired as the GEMM produces the rows;
   tables are stored [slice][core][row] so each slice's collective
   output is one contiguous block (BIR requires contiguous outputs).
 - per-edge records are pulled by src via batched SWDGE dma_gather:
   4 calls per batch (half-blocks of the lo/hi runs) so all 4 queues
   carry equal descriptor counts — the queues' ~15-20ns/packet issue
   rate is this kernel's roofline. Segment softmax is shift-free (tiny
   logits); leaky-relu is computed as x + relu(-0.8x) with the relu on
   the Scalar engine (a DVE tensor_scalar measured ~4.2us/op on HW, and
   the HW Lrelu activation has a hardwired 0.01 slope). Messages are
   aggregated per dst group with host-built fp8 one-hot matmuls in PSUM.
 - layer 2: the per-edge a_d2 broadcast (one-hot-transpose matmuls) is
   computed during layer 1 while the transposed one-hot tiles are still
   resident (saves reloading 23MB of fp8 one-hot). The elu's "-1" is
   folded out: r2' = (relu(t)+exp(-relu(-t)))@M4 differs from the true
   record by colsum(M4); the a_s2/a_d2 shift folds into the lrelu/exp
   biases and the h2 shift into the host-side bias (attention weights
   sum to 1). The layer-2 record AllGather moves compact 16B rows in 4
   slices fired under layer 1's tail, each expanded on-core through
   SBUF into the 256B-stride gather table (gather elems read only the
   first 12B of each row, so the pad bytes are never touched).
"""
import numpy as np
import ml_dtypes

import concourse.bass as bass
import concourse.bacc as bacc
import concourse.mybir as mybir
import concourse.tile as tile
from concourse.masks import make_identity

F32 = mybir.dt.float32
BF16 = mybir.dt.bfloat16
F8 = mybir.dt.float8e4
U8 = mybir.dt.uint8
I16 = mybir.dt.int16
NEG_SLOPE = 0.2
BF = ml_dtypes.bfloat16
F8NP = ml_dtypes.float8_e4m3

N_NODES = 65536
IN_FEAT = 768
N_CORES = 8
HEADS = 8
C1 = 16
OUT_FEAT = 2
SUP = 512
HALF = 32768  # int16 split point for gather tables
NQ = 4        # SWDGE queues (Q7 core pairs) to spread gathers over
NCC = 4       # collective row-slices (overlap with producer phase)
GB = 2        # dst groups per edge batch; chunks laid out [los..][his..]
              # per batch so each batch is two whole-block gather calls


class _Cfg:
    def __init__(self):
        self.N, self.IN, self.NC = N_NODES, IN_FEAT, N_CORES
        self.HEADS, self.C1, self.OUT, self.SUP = HEADS, C1, OUT_FEAT, SUP
        self.calo = []   # per-group lo-segment chunk counts
        self.cahi = []   # per-group hi-segment chunk counts

    @property
    def NL(self):
        return self.N // self.NC

    @property
    def G(self):
        return self.NL // 128

    @property
    def KT(self):
        return self.IN // 128

    @property
    def HID(self):
        return self.HEADS * self.C1

    @property
    def cg(self):
        return [a + b for a, b in zip(self.calo, self.cahi)]

    @property
    def Tlo(self):
        return sum(self.calo)

    @property
    def Thi(self):
        return sum(self.cahi)

    @property
    def T(self):
        return self.Tlo + self.Thi


def _wrap16(vals_by_slot, ncols):
    """vals_by_slot: int array indexed by slot j -> idx value.
    Returns [128, ncols] int16 with idx j at [j%16, j//16], replicated 8x."""
    a = np.zeros((16, ncols), np.int16)
    n = len(vals_by_slot)
    j = np.arange(n)
    a[j % 16, j // 16] = vals_by_slot
    return np.tile(a, (8, 1))


def _host_prep(cfg, x, edge_index, W1, att_src1, att_dst1, b1, Wp, bp,
               W2, att_src2, att_dst2, b2):
    N, NC, NL, G = cfg.N, cfg.NC, cfg.NL, cfg.G
    HID = cfg.HID
    x = np.asarray(x, np.float32)
    ei = np.asarray(edge_index)
    loops = np.arange(N, dtype=np.int64)
    src = np.concatenate([ei[0], loops]).astype(np.int64)
    dst = np.concatenate([ei[1], loops]).astype(np.int64)

    # the gathered tables are stored in [slice][core][row] order so each
    # AllGather row-slice lands as one contiguous output block (the BIR
    # verifier requires contiguous collective outputs); prow() maps a
    # global node id to its permuted table row, and the lo/hi int16 split
    # follows the PERMUTED row index (= local row < NL/2)
    QROWS = NL // NCC

    def prow(n):
        return (n % NL) // QROWS * (NC * QROWS) + (n // NL) * QROWS + n % QROWS

    hi_of = (src % NL) >= (NL // 2)
    # sort by (dst, hi) so each 128-dst group splits into lo/hi runs
    order = np.lexsort((hi_of, dst // 128))
    src, dst = src[order], dst[order]

    core_of = dst // NL
    core_starts = np.searchsorted(core_of, np.arange(NC + 1))

    # per-core per-group lo/hi counts
    nlo = np.zeros((NC, G), np.int64)
    nhi = np.zeros((NC, G), np.int64)
    for k in range(NC):
        s, e = core_starts[k], core_starts[k + 1]
        g = (dst[s:e] - k * NL) // 128
        hi = ((src[s:e] % NL) >= (NL // 2)).astype(np.int64)
        nlo[k] = np.bincount(g, weights=1 - hi, minlength=G)
        nhi[k] = np.bincount(g, weights=hi, minlength=G)
    calo = np.maximum((-(-nlo.astype(np.int64) // 128)).max(axis=0), 1)
    cahi = np.maximum((-(-nhi.astype(np.int64) // 128)).max(axis=0), 1)
    cfg.calo = [int(c) for c in calo]
    cfg.cahi = [int(c) for c in cahi]
    cg = calo + cahi
    offlo = np.concatenate([[0], np.cumsum(calo)])
    offhi = np.concatenate([[0], np.cumsum(cahi)])
    off = np.concatenate([[0], np.cumsum(cg)])
    Tlo, Thi, T = int(calo.sum()), int(cahi.sum()), int(cg.sum())

    W1 = np.asarray(W1, np.float32)
    Wp = np.asarray(Wp, np.float32)
    As = np.zeros((HID, cfg.HEADS), np.float32)
    Ad = np.zeros((HID, cfg.HEADS), np.float32)
    hh = np.repeat(np.arange(cfg.HEADS), cfg.C1)
    As[np.arange(HID), hh] = np.asarray(att_src1, np.float32).ravel()
    Ad[np.arange(HID), hh] = np.asarray(att_dst1, np.float32).ravel()
    WBIG = np.concatenate([W1, W1 @ As, W1 @ Ad, Wp], axis=1)  # [IN, 272]

    W2 = np.asarray(W2, np.float32)
    M4 = np.concatenate(
        [W2, W2 @ np.asarray(att_src2, np.float32).T,
         W2 @ np.asarray(att_dst2, np.float32).T], axis=1)     # [HID, 4]
    BC1 = (np.asarray(b1, np.float32) + np.asarray(bp, np.float32))[None, :]
    B2 = np.asarray(b2, np.float32)
    # elu(-1) fold: r2' = h2sum@M4 = r2_true + colsum(M4). as2/ad2 shift is
    # folded into the layer-2 Lrelu bias; h2 shift into the host bias.
    csum = M4.sum(axis=0)                                      # [4]
    d = -(csum[2] + csum[3])          # shift to apply before layer-2 lrelu
    # lrelu(x+d) computed as (x+d) + relu(-0.8(x+d)); col0 biases the Exp,
    # col1 biases the Relu (HW Lrelu has a hardwired 0.01 slope, unusable)
    DLT = np.array([[d, -0.8 * d]], np.float32)
    B2eff = B2 - csum[0:2]

    in_maps = []
    for k in range(NC):
        s, e = core_starts[k], core_starts[k + 1]
        sk, dk = src[s:e], dst[s:e]
        gk = (dk - k * NL) // 128
        hik = (sk % NL) >= (NL // 2)
        # rank within (group, seg): edges are sorted by (group, hi) so
        # positions within each (g, seg) run are consecutive
        pos = np.arange(e - s)
        seg_key = gk * 2 + hik
        seg_start = np.concatenate([[0], np.cumsum(np.bincount(
            seg_key, minlength=2 * G))])
        r_in_seg = pos - seg_start[seg_key]
        # chunk position within the batch: [lo g0][lo g1][hi g0][hi g1]
        g0b = (gk // GB) * GB               # first group of the edge's batch
        lo_base = np.where(gk == g0b, 0, calo[g0b])
        lsum = calo[g0b] + calo[g0b + 1]
        hi_base = lsum + np.where(gk == g0b, 0, cahi[g0b])
        cpos = np.where(hik, hi_base, lo_base) + r_in_seg // 128
        chunk = off[g0b] + cpos             # global chunk index
        lane = r_in_seg % 128

        # lo/hi gather slot (position within that segment's idx stream)
        slot_lo = (offlo[gk] * 128 + r_in_seg)[~hik]
        slot_hi = (offhi[gk] * 128 + r_in_seg)[hik]
        vals = np.zeros(Tlo * 128, np.int64)
        vals[slot_lo] = prow(sk[~hik])
        SRCLO = _wrap16(vals, 8 * Tlo)
        vals = np.zeros(Thi * 128, np.int64)
        vals[slot_hi] = prow(sk[hik]) - HALF
        SRCHI = _wrap16(vals, 8 * Thi)
        # one-hot tiles (static graph): chunk c col-block holds
        # oh[lane, dstpos] = 1 for each real edge; padding rows all-zero,
        # plus the transposed tiles for dst->edge broadcasts (a_d terms)
        dpos = (dk - k * NL) % 128
        OH = np.zeros((128, T * 128), F8NP)
        OH[lane, chunk * 128 + dpos] = 1
        OHT = np.zeros((128, T * 128), F8NP)
        OHT[dpos, chunk * 128 + lane] = 1

        in_maps.append({
            "XT": np.ascontiguousarray(
                x[k * NL:(k + 1) * NL].T).astype(BF),
            "WBIG": WBIG.astype(BF), "M4": M4, "BC1": BC1, "DLT": DLT,
            "SRCLO": SRCLO, "SRCHI": SRCHI, "OH": OH, "OHT": OHT,
        })
    return cfg, in_maps, B2eff


def _unshard(cfg, outs, B2eff):
    parts = []
    for k in range(cfg.NC):
        o = outs[k]["OUT"]                       # [4, G*128]
        num = o[0:2].reshape(2, cfg.G, 128)
        den = o[2].reshape(cfg.G, 128)
        r = (num / den[None]).transpose(1, 2, 0).reshape(cfg.NL, 2)
        parts.append(r + B2eff[None, :])
    return np.concatenate(parts, axis=0).astype(np.float32)


MAX_GCH = 32  # chunks (x128 idxs) per dma_gather call


def _gather(nc, out_tile, out_col0, in_ap, idx_tile, idx_col0, nchunks, elem,
            stride_bytes, queue_num):
    """Batched dma_gather, split into <=MAX_GCH-chunk calls on queue_num.
    out rows: [128, nchunks, elem] at out_tile cols out_col0*elem;
    idx cols: idx_tile[:, 8*idx_col0 : 8*(idx_col0+nchunks)]."""
    eng = nc.gpsimd
    insts = []
    for c0 in range(0, nchunks, MAX_GCH):
        nch = min(MAX_GCH, nchunks - c0)
        o = out_tile[:, (out_col0 + c0) * elem:(out_col0 + c0 + nch) * elem]
        o = o.rearrange("p (n e) -> p n e", e=elem)
        ix = idx_tile[:, 8 * (idx_col0 + c0):8 * (idx_col0 + c0 + nch)]
        insts.append(eng.add_instruction(
            mybir.InstDMAGatherAnt(
                name=nc.get_next_instruction_name(),
                ins=[*eng.lower_ap_dma(in_ap, for_custom_bir_dma=True),
                     eng.lower_ap(ix),
                     eng.lower_val_access(eng.to_reg(nch * 128))],
                outs=[eng.lower_ap(o)],
                transpose=False,
                num_idxs=nch * 128,
                elem_size=elem,
                stride_bytes_256=stride_bytes // 256,
                gen_mode=0,
                single_packet=False,
                queue_num=queue_num,
            )))
    return insts


def _build(cfg):
    NC = cfg.NC
    NL, G, KT = cfg.NL, cfg.G, cfg.KT
    H, C1_, HID, OUT = cfg.HEADS, cfg.C1, cfg.HID, cfg.OUT
    REC = HID + 2 * H        # 144 (gemm out: h|a_s|a_d)
    GREC = HID + H           # 136 (gathered: h|a_s)
    RBYTES = HID + 2 * H     # 144B gathered record: h fp8 | a_s bf16
    TBYTES = 256             # T1main row stride in bytes
    R2W = 64                 # f32 per R2main row (256B)
    SUP_ = cfg.SUP
    calo, cahi, cg = cfg.calo, cfg.cahi, cfg.cg
    Tlo, Thi, T = cfg.Tlo, cfg.Thi, cfg.T
    offlo = [0]
    for c in calo:
        offlo.append(offlo[-1] + c)
    offhi = [0]
    for c in cahi:
        offhi.append(offhi[-1] + c)
    off = [0]
    for c in cg:
        off.append(off[-1] + c)

    QROWS = NL // NCC        # rows per collective slice (2048)
    QG = G // NCC            # groups per collective slice (16)

    qrot = [0]

    def nextq():
        q = qrot[0]
        qrot[0] = (q + 1) % NQ
        return q

    nc = bacc.Bacc("TRN2", target_bir_lowering=False, debug=False,
                   num_devices=NC, num_swdge_queues=NQ)
    XT = nc.dram_tensor("XT", [cfg.IN, NL], BF16, kind="ExternalInput")
    WBIGd = nc.dram_tensor("WBIG", [cfg.IN, REC + HID], BF16, kind="ExternalInput")
    M4d = nc.dram_tensor("M4", [HID, 4], F32, kind="ExternalInput")
    BC1d = nc.dram_tensor("BC1", [1, HID], F32, kind="ExternalInput")
    DLTd = nc.dram_tensor("DLT", [1, 2], F32, kind="ExternalInput")
    SRCLOd = nc.dram_tensor("SRCLO", [128, 8 * Tlo], I16, kind="ExternalInput")
    SRCHId = nc.dram_tensor("SRCHI", [128, 8 * Thi], I16, kind="ExternalInput")
    OHd = nc.dram_tensor("OH", [128, T * 128], F8, kind="ExternalInput")
    OHTd = nc.dram_tensor("OHT", [128, T * 128], F8, kind="ExternalInput")
    OUTd = nc.dram_tensor("OUT", [4, G * 128], F32, kind="ExternalOutput")

    with tile.TileContext(nc) as tc:
        with (
            tc.tile_pool(name="dram", bufs=1, space="DRAM") as dram,
            tc.tile_pool(name="const", bufs=1) as cb,
            tc.tile_pool(name="persist", bufs=1) as pp,
        ):
            T1locq = [dram.tile([QROWS, TBYTES], U8, name=f"T1locq{q}")
                      for q in range(NCC)]
            T1main = dram.tile([cfg.N, TBYTES], U8)
            R2locq = [dram.tile([QROWS, 4], F32, name=f"R2locq{q}")
                      for q in range(NCC)]
            R2comp = dram.tile([cfg.N, 4], F32)
            R2allcP = dram.tile([cfg.N, R2W], F32)

            ident = cb.tile([128, 128], F32)
            make_identity(nc, ident[:])
            wb_sb = []
            for kk in range(KT):
                t = cb.tile([128, REC + HID], BF16, tag=f"wb{kk}", name=f"wb{kk}")
                nc.sync.dma_start(t[:], WBIGd[kk * 128:(kk + 1) * 128, :])
                wb_sb.append(t)
            m4_sb = cb.tile([HID, 4], F32)
            nc.sync.dma_start(m4_sb[:], M4d[:])
            ones1 = cb.tile([1, 128], F32)
            nc.vector.memset(ones1[:], 1.0)
            bc1row = cb.tile([1, HID], F32)
            nc.sync.dma_start(bc1row[:], BC1d[:])
            dltrow = cb.tile([1, 2], F32)
            nc.sync.dma_start(dltrow[:], DLTd[:])
            with tc.tile_pool(name="bpsum", bufs=1, space="PSUM") as bps:
                bp1 = bps.tile([128, HID], F32)
                nc.tensor.matmul(bp1[:], lhsT=ones1[:], rhs=bc1row[:], start=True, stop=True)
                BC1T = cb.tile([128, HID], F32)
                nc.vector.tensor_copy(BC1T[:], bp1[:])
                bpd = bps.tile([128, 2], F32, tag="bpd", name="bpd")
                nc.tensor.matmul(bpd[:], lhsT=ones1[:], rhs=dltrow[:], start=True, stop=True)
                DLTT = cb.tile([128, 2], F32)
                nc.vector.tensor_copy(DLTT[:], bpd[:])

            p_sb = pp.tile([128, G * HID], F32)
            ad_sb = pp.tile([128, G * H], BF16)
            r2stage = pp.tile([128, G * 4], F32)
            adp2all = pp.tile([128, T], F32)
            ad2bf = pp.tile([128, G], BF16)
            outstage = pp.tile([4, G * 128], F32)
            nc.vector.memset(outstage[:], 0.0)
            # gather index tables stay resident (shared by both layers)
            silo_all = pp.tile([128, 8 * Tlo], I16)
            nc.sync.dma_start(silo_all[:], SRCLOd[:])
            sihi_all = pp.tile([128, 8 * Thi], I16)
            nc.sync.dma_start(sihi_all[:], SRCHId[:])

            # ---------------- phase A: GEMM (+ T1 AllGather slices)
            n_sup = NL // SUP_
            m_per = SUP_ // 128
            sup_per_q = n_sup // NCC
            ccs1 = []
            with (
                tc.tile_pool(name="xts", bufs=2 * KT) as xp,
                tc.tile_pool(name="gpsum", bufs=3, space="PSUM") as gps,
                tc.tile_pool(name="grec", bufs=3) as grp,
            ):
                for s in range(n_sup):
                    xts = []
                    for kk in range(KT):
                        t = xp.tile([128, SUP_], BF16, tag="xts", name="xts")
                        nc.sync.dma_start(
                            t[:], XT[kk * 128:(kk + 1) * 128, s * SUP_:(s + 1) * SUP_])
                        xts.append(t)
                    for m in range(m_per):
                        gm = s * m_per + m
                        q = gm // QG
                        r0 = (gm % QG) * 128
                        ps = gps.tile([128, REC + HID], F32, tag="gp", name="gp")
                        for kk in range(KT):
                            nc.tensor.matmul(
                                ps[:], lhsT=xts[kk][:, m * 128:(m + 1) * 128],
                                rhs=wb_sb[kk][:], start=(kk == 0), stop=(kk == KT - 1))
                        rec = grp.tile([128, TBYTES], U8, tag="rec", name="rec")
                        nc.vector.tensor_copy(
                            rec[:, 0:HID].bitcast(F8), ps[:, 0:HID])
                        nc.vector.tensor_copy(
                            rec[:, HID:RBYTES].bitcast(BF16), ps[:, HID:GREC])
                        nc.vector.tensor_copy(
                            ad_sb[:, gm * H:(gm + 1) * H], ps[:, GREC:REC])
                        nc.vector.tensor_copy(
                            p_sb[:, gm * HID:(gm + 1) * HID], ps[:, REC:REC + HID])
                        nc.sync.dma_start(T1locq[q][r0:r0 + 128, :], rec[:])
                    # fire the AllGather slice as soon as its rows are done
                    # (T1main is in [slice][core][row] order so each slice's
                    # output is one contiguous block)
                    if (s + 1) % sup_per_q == 0:
                        q = (s + 1) // sup_per_q - 1
                        out_ap = bass.AP(
                            T1main.tensor, q * NC * QROWS * TBYTES,
                            [[TBYTES, NC * QROWS], [1, TBYTES]])
                        ccs1.append(nc.gpsimd.collective_compute(
                            "AllGather", mybir.AluOpType.bypass,
                            replica_groups=[list(range(NC))],
                            ins=[T1locq[q].opt()], outs=[out_ap.opt()]))

            T1lo_h = T1main[:][0:HALF, 0:RBYTES]
            T1hi_h = T1main[:][HALF:cfg.N, 0:RBYTES]

            # ---------------- phase C: layer-1 edge pass + layer-2 prep
            assert G % GB == 0
            ccs2 = []

            def _cpos(gs):
                """Batch-relative chunk positions per group for the
                [lo g0][lo g1][hi g0][hi g1] batch layout."""
                c0, h0 = calo[gs[0]], cahi[gs[0]]
                c1, h1 = calo[gs[1]], cahi[gs[1]]
                L = c0 + c1
                return {
                    gs[0]: list(range(0, c0)) + list(range(L, L + h0)),
                    gs[1]: list(range(c0, L)) + list(range(L + h0, L + h0 + h1)),
                }

            SUBR = 2048  # rows per expansion sub-chunk (16 rows/partition)
            rp_insts = []
            exp_pend = []  # deferred (q, sb) expansion sub-chunks

            def _expand_r2(q, sb):
                # expand one compact sub-chunk into 256B-stride gather rows
                # via SBUF (the 12B gather elems never read the pad bytes, so
                # no zero-fill needed); contiguous DMAs = 128 descriptors
                r0 = q * NC * QROWS + sb * SUBR
                cst = xpn.tile([128, SUBR // 128 * 4], F32,
                               tag="cst", name="cst")
                ld = nc.sync.dma_start(
                    cst[:].rearrange("p (g r) -> p g r", r=4),
                    R2comp[:][r0:r0 + SUBR, :].rearrange(
                        "(p g) r -> p g r", p=128))
                tile.add_dep_helper(ld.ins, ccs2[q].ins, sync=True,
                                    reason="xp")
                est = xpn.tile([128, SUBR // 128 * R2W], F32,
                               tag="est", name="est")
                nc.vector.tensor_copy(
                    bass.AP(est.tensor, est[:].offset,
                            [est[:].ap[0], [R2W, SUBR // 128], [1, 4]]),
                    cst[:].rearrange("p (g r) -> p g r", r=4))
                rp_insts.append(nc.sync.dma_start(
                    R2allcP[:][r0:r0 + SUBR, :].rearrange(
                        "(p g) r -> p g r", p=128), est[:].rearrange(
                            "p (g r) -> p g r", r=R2W)))

            def _fire_r2(q):
                r2v = r2stage[:, q * QG * 4:(q + 1) * QG * 4].rearrange(
                    "p (g r) -> p g r", r=4)
                nc.sync.dma_start(
                    R2locq[q][:].rearrange("(g p) r -> p g r", p=128), r2v)
                out_ap = bass.AP(
                    R2comp.tensor, q * NC * QROWS * 4,
                    [[4, NC * QROWS], [1, 4]])
                ccs2.append(nc.gpsimd.collective_compute(
                    "AllGather", mybir.AluOpType.bypass,
                    replica_groups=[list(range(NC))],
                    ins=[R2locq[q].opt()], outs=[out_ap.opt()]))
                exp_pend.extend((q, sb) for sb in range(NC * QROWS // SUBR))

            with (
                tc.tile_pool(name="xpnd", bufs=2) as xpn,
                tc.tile_pool(name="erec", bufs=3) as ep,
                tc.tile_pool(name="ework", bufs=2) as ew,
                tc.tile_pool(name="escall", bufs=3) as esc,
                tc.tile_pool(name="eoh", bufs=2) as eoh,
                tc.tile_pool(name="epsum", bufs=2, space="PSUM") as eps,
                tc.tile_pool(name="apsum", bufs=2, space="PSUM") as aps,
                tc.tile_pool(name="tpsum", bufs=2, space="PSUM") as tps,
            ):
                for g0 in range(0, G, GB):
                    gs = list(range(g0, min(g0 + GB, G)))
                    nb = off[gs[-1] + 1] - off[g0]
                    base0 = off[g0]
                    oht = eoh.tile([128, nb * 128], F8, tag="oht", name="oht")
                    nc.sync.dma_start(
                        oht[:], OHd[:, base0 * 128:(base0 + nb) * 128])
                    ohtT = eoh.tile([128, nb * 128], F8, tag="ohtT", name="ohtT")
                    nc.sync.dma_start(
                        ohtT[:], OHTd[:, base0 * 128:(base0 + nb) * 128])
                    # one whole-block gather call per (batch, lo/hi table)
                    cpos = _cpos(gs)
                    nlo_b = calo[gs[0]] + calo[gs[1]]
                    nhi_b = nb - nlo_b
                    hall = ep.tile([128, nb * RBYTES], U8, tag="hall", name="hall")
                    # two half-block calls per table so each batch spreads
                    # its descriptors across all 4 SWDGE queues
                    hlo = (nlo_b + 1) // 2
                    hhi = (nhi_b + 1) // 2
                    for c0, nch, tbl, idxt, ic0 in (
                            (0, hlo, T1lo_h, silo_all, offlo[gs[0]]),
                            (hlo, nlo_b - hlo, T1lo_h, silo_all,
                             offlo[gs[0]] + hlo),
                            (nlo_b, hhi, T1hi_h, sihi_all, offhi[gs[0]]),
                            (nlo_b + hhi, nhi_b - hhi, T1hi_h, sihi_all,
                             offhi[gs[0]] + hhi)):
                        for gi in _gather(nc, hall[:], c0, tbl, idxt[:], ic0,
                                          nch, RBYTES, TBYTES, nextq()):
                            for cc in ccs1:
                                tile.add_dep_helper(gi.ins, cc.ins, sync=True,
                                                    reason="ag1")
                    # batch-wide per-edge a_d via one-hot-transpose matmuls;
                    # tail region [nb*H:nb*H+nb] is layer-2's a_d2 (filled
                    # after ps4 below, same PSUM bank)
                    adp = aps.tile([128, nb * H + nb], F32, tag="adp", name="adp")
                    for g in gs:
                        for c in cpos[g]:
                            nc.tensor.matmul(
                                adp[:, c * H:(c + 1) * H],
                                lhsT=ohtT[:, c * 128:(c + 1) * 128],
                                rhs=ad_sb[:, g * H:(g + 1) * H],
                                start=True, stop=True)
                    # batch-wide softmax numerators: e = lrelu(a_s + a_d);
                    # exp on Scalar engine straight into the scall tile
                    scall = esc.tile([128, nb * GREC], BF16, tag="scall", name="scall")
                    as_ap = bass.AP(
                        hall.tensor, hall[:].offset + HID,
                        [hall[:].ap[0], [RBYTES, nb], [1, 2 * H]]).bitcast(BF16)
                    ad_ap = bass.AP(
                        adp.tensor, adp[:].offset,
                        [adp[:].ap[0], [H, nb], [1, H]])
                    epre = ew.tile([128, nb * H], F32, tag="epre", name="epre")
                    nc.vector.tensor_tensor(
                        out=epre[:].rearrange("p (n h) -> p n h", h=H),
                        in0=as_ap, in1=ad_ap, op=mybir.AluOpType.add)
                    # lrelu(x) = x + relu(-0.8x); HW Lrelu slope is fixed 0.01
                    rneg = ew.tile([128, nb * H], F32, tag="rneg", name="rneg")
                    nc.scalar.activation(rneg[:], epre[:],
                                         mybir.ActivationFunctionType.Relu,
                                         scale=-(1.0 - NEG_SLOPE))
                    lrm = ew.tile([128, nb * H], F32, tag="lrm", name="lrm")
                    nc.vector.tensor_add(lrm[:], epre[:], rneg[:])
                    ex_ap = bass.AP(
                        scall.tensor, scall[:].offset + HID,
                        [scall[:].ap[0], [GREC, nb], [1, H]])
                    nc.scalar.activation(
                        ex_ap, lrm[:].rearrange("p (n h) -> p n h", h=H),
                        mybir.ActivationFunctionType.Exp)
                    # scaled messages (bf16 h x bf16 ex -> bf16)
                    out4 = bass.AP(
                        scall.tensor, scall[:].offset,
                        [scall[:].ap[0], [GREC, nb], [C1_, H], [1, C1_]])
                    in04 = bass.AP(
                        hall.tensor, hall[:].offset,
                        [hall[:].ap[0], [RBYTES, nb], [C1_, H], [1, C1_]]).bitcast(F8)
                    in14 = bass.AP(
                        scall.tensor, scall[:].offset + HID,
                        [scall[:].ap[0], [GREC, nb], [1, H], [0, C1_]])
                    nc.vector.tensor_tensor(
                        out=out4, in0=in04, in1=in14, op=mybir.AluOpType.mult)

                    for g in gs:
                        psg = eps.tile([128, GREC], F32, tag="psg", name="psg")
                        for j, c in enumerate(cpos[g]):
                            nc.tensor.matmul(
                                psg[:], lhsT=oht[:, c * 128:(c + 1) * 128],
                                rhs=scall[:, c * GREC:(c + 1) * GREC],
                                start=(j == 0), stop=(j == len(cpos[g]) - 1))
                        # normalize + residual + elu(+1) -> h2sum -> r2 records
                        rec8 = ew.tile([128, H], F32, tag="rec8", name="rec8")
                        nc.vector.reciprocal(rec8[:], psg[:, HID:GREC])
                        t1 = ew.tile([128, HID], F32, tag="t1", name="t1")
                        nc.vector.tensor_tensor(
                            out=t1[:].rearrange("p (h c) -> p h c", h=H),
                            in0=psg[:, 0:HID].rearrange("p (h c) -> p h c", h=H),
                            in1=rec8[:].to_broadcast([128, H, C1_]),
                            op=mybir.AluOpType.mult)
                        nc.vector.tensor_add(t1[:], t1[:], p_sb[:, g * HID:(g + 1) * HID])
                        nc.vector.tensor_add(t1[:], t1[:], BC1T[:])
                        tmin = ew.tile([128, HID], F32, tag="tmin", name="tmin")
                        nc.scalar.activation(tmin[:], t1[:],
                                             mybir.ActivationFunctionType.Relu,
                                             scale=-1.0)
                        texp = ew.tile([128, HID], F32, tag="texp", name="texp")
                        nc.scalar.activation(texp[:], tmin[:],
                                             mybir.ActivationFunctionType.Exp,
                                             scale=-1.0)
                        tmax = ew.tile([128, HID], F32, tag="tmax", name="tmax")
                        nc.scalar.activation(tmax[:], t1[:],
                                             mybir.ActivationFunctionType.Relu)
                        h2sum = ew.tile([128, HID], F32, tag="h2sum", name="h2sum")
                        nc.vector.tensor_add(h2sum[:], texp[:], tmax[:])
                        pst = tps.tile([128, HID], F32, tag="pst", name="pst")
                        nc.tensor.transpose(pst[:], h2sum[:], ident[:])
                        tT = ew.tile([128, HID], F32, tag="tT", name="tT")
                        nc.vector.tensor_copy(tT[:], pst[:])
                        ps4 = tps.tile([128, 4], F32, tag="ps4", name="ps4")
                        nc.tensor.matmul(ps4[:], lhsT=tT[:], rhs=m4_sb[:],
                                         start=True, stop=True)
                        nc.vector.tensor_copy(r2stage[:, g * 4:(g + 1) * 4], ps4[:])
                        # layer-2 prep while ohtT is resident: a_d2 broadcast
                        nc.vector.tensor_copy(ad2bf[:, g:g + 1], ps4[:, 3:4])
                        for c in cpos[g]:
                            nc.tensor.matmul(
                                adp[:, nb * H + c:nb * H + c + 1],
                                lhsT=ohtT[:, c * 128:(c + 1) * 128],
                                rhs=ad2bf[:, g:g + 1],
                                start=True, stop=True)
                    nc.vector.tensor_copy(
                        adp2all[:, base0:base0 + nb], adp[:, nb * H:nb * H + nb])

                    # fire R2 AllGather slices 2 batches after their groups
                    # retire so the queue-head waits are already satisfied
                    # (an early sem-wait would stall the issuing queues)
                    gend = gs[-1] + 1
                    while (len(ccs2) < NCC
                           and gend >= QG * (len(ccs2) + 1) + 2 * GB):
                        _fire_r2(len(ccs2))
                    if exp_pend:
                        _expand_r2(*exp_pend.pop(0))
                while len(ccs2) < NCC:
                    _fire_r2(len(ccs2))
                while exp_pend:
                    _expand_r2(*exp_pend.pop(0))

            R2lo_h = R2allcP[:][0:HALF, 0:3]
            R2hi_h = R2allcP[:][HALF:cfg.N, 0:3]

            # ---------------- phase E: layer-2 edge pass
            with (
                tc.tile_pool(name="e2rec", bufs=3) as ep2,
                tc.tile_pool(name="e2work", bufs=3) as ew2,
                tc.tile_pool(name="e2sc", bufs=3) as esc2,
                tc.tile_pool(name="e2oh", bufs=3) as eoh2,
                tc.tile_pool(name="e2psum", bufs=2, space="PSUM") as eps2,
            ):
                for g0 in range(0, G, GB):
                    gs = list(range(g0, min(g0 + GB, G)))
                    nb = off[gs[-1] + 1] - off[g0]
                    base0 = off[g0]
                    oht2 = eoh2.tile([128, nb * 128], F8, tag="oht2", name="oht2")
                    nc.sync.dma_start(
                        oht2[:], OHd[:, base0 * 128:(base0 + nb) * 128])
                    cpos = _cpos(gs)
                    nlo_b = calo[gs[0]] + calo[gs[1]]
                    nhi_b = nb - nlo_b
                    rall = ep2.tile([128, nb * 3], F32, tag="rall", name="rall")
                    hlo = (nlo_b + 1) // 2
                    hhi = (nhi_b + 1) // 2
                    for c0, nch, tbl, idxt, ic0 in (
                            (0, hlo, R2lo_h, silo_all, offlo[gs[0]]),
                            (hlo, nlo_b - hlo, R2lo_h, silo_all,
                             offlo[gs[0]] + hlo),
                            (nlo_b, hhi, R2hi_h, sihi_all, offhi[gs[0]]),
                            (nlo_b + hhi, nhi_b - hhi, R2hi_h, sihi_all,
                             offhi[gs[0]] + hhi)):
                        for gi in _gather(nc, rall[:], c0, tbl, idxt[:], ic0,
                                          nch, 3, R2W * 4, nextq()):
                            for st in rp_insts:
                                tile.add_dep_helper(gi.ins, st.ins, sync=True,
                                                    reason="rp2")
                    sc2 = esc2.tile([128, nb * 3], BF16, tag="sc2", name="sc2")
                    as2_ap = bass.AP(
                        rall.tensor, rall[:].offset + 2,
                        [rall[:].ap[0], [3, nb], [1, 1]])
                    ad2_ap = bass.AP(
                        adp2all.tensor, adp2all[:].offset + base0,
                        [adp2all[:].ap[0], [1, nb], [1, 1]])
                    epre = ew2.tile([128, nb], F32, tag="ep2", name="ep2")
                    nc.vector.tensor_tensor(
                        out=epre[:].rearrange("p (n h) -> p n h", h=1),
                        in0=as2_ap, in1=ad2_ap, op=mybir.AluOpType.add)
                    # lrelu(x+d) = (x+d) + relu(-0.8(x+d)); the relu bias is
                    # -0.8d (DLTT col1), the +d rides on the Exp bias (col0)
                    rneg = ew2.tile([128, nb], F32, tag="rn2", name="rn2")
                    nc.scalar.activation(rneg[:], epre[:],
                                         mybir.ActivationFunctionType.Relu,
                                         scale=-(1.0 - NEG_SLOPE),
                                         bias=DLTT[:, 1:2])
                    lrm = ew2.tile([128, nb], F32, tag="lrm2", name="lrm2")
                    nc.vector.tensor_add(lrm[:], epre[:], rneg[:])
                    ex_ap = bass.AP(
                        sc2.tensor, sc2[:].offset + 2,
                        [sc2[:].ap[0], [3, nb], [1, 1]])
                    nc.scalar.activation(
                        ex_ap, lrm[:].rearrange("p (n h) -> p n h", h=1),
                        mybir.ActivationFunctionType.Exp, bias=DLTT[:, 0:1])
                    out4 = bass.AP(
                        sc2.tensor, sc2[:].offset,
                        [sc2[:].ap[0], [3, nb], [1, 1], [1, OUT]])
                    in04 = bass.AP(
                        rall.tensor, rall[:].offset,
                        [rall[:].ap[0], [3, nb], [1, 1], [1, OUT]])
                    in14 = bass.AP(
                        sc2.tensor, sc2[:].offset + 2,
                        [sc2[:].ap[0], [3, nb], [1, 1], [0, OUT]])
                    nc.vector.tensor_tensor(
                        out=out4, in0=in04, in1=in14, op=mybir.AluOpType.mult)

                    for g in gs:
                        ps2 = eps2.tile([3, 128], F32, tag="ps2", name="ps2")
                        for j, c in enumerate(cpos[g]):
                            nc.tensor.matmul(
                                ps2[:], lhsT=sc2[:, c * 3:(c + 1) * 3],
                                rhs=oht2[:, c * 128:(c + 1) * 128],
                                start=(j == 0), stop=(j == len(cpos[g]) - 1))
                        nc.vector.tensor_copy(
                            outstage[0:3, g * 128:(g + 1) * 128], ps2[:])
                nc.sync.dma_start(OUTd[:], outstage[:])

    nc.compile()
    return nc


_CACHE = {}


def kernel(x, edge_index, W1, att_src1, att_dst1, b1, Wp, bp,
           W2, att_src2, att_dst2, b2, _trace=False):
    from concourse.bass_utils import run_bass_kernel_spmd
    cfg = _Cfg()
    cfg, in_maps, B2eff = _host_prep(
        cfg, x, edge_index, W1, att_src1, att_dst1, b1, Wp, bp,
        W2, att_src2, att_dst2, b2)
    key = (tuple(cfg.calo), tuple(cfg.cahi))
    if key not in _CACHE:
        _CACHE[key] = _build(cfg)
    nc = _CACHE[key]
    res = run_bass_kernel_spmd(
        nc, in_maps, core_ids=list(range(cfg.NC)), trace=_trace)
    out = _unshard(cfg, [res.results[k] for k in range(cfg.NC)], B2eff)
    kernel.last_exec_time_ns = res.exec_time_ns
    return out
